# revision 20
# baseline (speedup 1.0000x reference)
"""Causal self-attention Trainium2 kernel (8 NeuronCores, bf16 compute).

Sharding: core c -> batch b = c//4, head group hg = c%4 (4 heads each).
Each core computes its heads' QKV projections, causal attention, and a
partial output projection yt[d, t] (transposed). Host sums the 4 partials
per batch, transposes, and adds b_proj.

Device dataflow per core (software-pipelined across heads):
  qkv(h) : per span/proj: PSUM = W.T @ xT chunks -> ACT bias -> QT/KT/VT
           VT 128-blocks transposed to natural V via DMA XBAR transpose
  attn(h): per q-span (512): for each k-block kj:
             ST[k,q] = KT_blk.T @ QT_span   (PE, scores transposed)
             += causal mask on diagonal blocks (DVE)
             PT = exp(scale*ST)             (ACT, bf16, unnormalized)
             acc[128,q] += PT               (DVE, f32)
             OT[hd,q] += V_blk.T @ PT       (PE, lagged)
           sum = partition_all_reduce(acc)  (GPSIMD)
           recip (DVE), OT_sbuf = OT * recip (DVE, bf16)
  proj   : yt[dc,t] = sum_h Wp_h.T @ OT_h -> chunked DMA out
  Interleave: attn(h-1) units are woven between qkv(h) units, and
  attn(3) between proj units, so the ACT-bound exp chain never stalls
  the PE.
"""
import numpy as np

B, S, D, H = 2, 2048, 2048, 16
HD = 128
NCORES = 8
HPC = H // (NCORES // B)     # heads per core = 4
NEG = -1e9


def build_nc(S=S, D=D, nh=HPC, span=512):
    import concourse.mybir as mybir
    from concourse import bacc
    from concourse import bass_isa
    from concourse.tile import TileContext

    f32 = mybir.dt.float32
    bf16 = mybir.dt.bfloat16
    KT = D // 128          # contraction tiles for qkv
    TT = S // 128          # token tiles
    NS = S // span         # q spans
    KPS = span // 128      # k-blocks per span
    DC = D // 128
    scale = float(HD) ** -0.5
    LAG = 2

    nc = bacc.Bacc("TRN2", target_bir_lowering=False, debug=False)
    x_d = nc.dram_tensor("xt", [D, S], bf16, kind="ExternalInput").ap()
    wq_d = nc.dram_tensor("wqkv", [3 * nh * 128, D], bf16,
                          kind="ExternalInput").ap()
    bq_d = nc.dram_tensor("bqkv", [128, 3 * nh], f32, kind="ExternalInput").ap()
    wp_d = nc.dram_tensor("wproj", [nh * 128, D], bf16,
                          kind="ExternalInput").ap()
    tm_d = nc.dram_tensor("trimaskT", [128, 128], f32,
                          kind="ExternalInput").ap()
    id_d = nc.dram_tensor("identb", [128, 128], bf16, kind="ExternalInput").ap()
    yt_d = nc.dram_tensor("yt", [D, S], f32, kind="ExternalOutput").ap()

    Act = mybir.ActivationFunctionType
    Alu = mybir.AluOpType

    with TileContext(nc) as tc:
        from contextlib import ExitStack
        with ExitStack() as ctx:
            res = ctx.enter_context(tc.tile_pool(name="res", bufs=1))
            w_p = ctx.enter_context(tc.tile_pool(name="w", bufs=6))
            wp_p = ctx.enter_context(tc.tile_pool(name="wp", bufs=nh))
            qk_p = ctx.enter_context(tc.tile_pool(name="qk", bufs=2))
            v_p = ctx.enter_context(tc.tile_pool(name="v", bufs=2))
            pt_p = ctx.enter_context(tc.tile_pool(name="pt", bufs=4))
            sm_p = ctx.enter_context(tc.tile_pool(name="sm", bufs=2))
            yc_p = ctx.enter_context(tc.tile_pool(name="yc", bufs=4))
            ps_mm = ctx.enter_context(
                tc.tile_pool(name="ps_mm", bufs=2, space="PSUM"))
            ps_o = ctx.enter_context(
                tc.tile_pool(name="ps_o", bufs=2, space="PSUM"))
            ps_st = ctx.enter_context(
                tc.tile_pool(name="ps_st", bufs=4, space="PSUM"))

            # constants (issued on the ACT queue; SP queue is for x panels)
            trimaskT = res.tile([128, 128], f32, tag="trimaskT")
            identb = res.tile([128, 128], bf16, tag="identb")
            bq = res.tile([128, 3 * nh], f32, tag="bq")
            nc.scalar.dma_start(trimaskT, tm_d)
            nc.scalar.dma_start(identb, id_d)
            nc.scalar.dma_start(bq, bq_d)

            wt = {}

            def issue_w(h, eng=None, halves=1):
                for p in range(3):
                    t = w_p.tile([128, D], bf16, tag="w", name=f"w{h}_{p}")
                    r0 = (p * nh + h) * 128
                    hD = D // halves
                    for q in range(halves):
                        (eng or nc.sync).dma_start(
                            t[:, q * hD:(q + 1) * hD],
                            wq_d[r0:r0 + 128, q * hD:(q + 1) * hD])
                    wt[(h, p)] = t

            issue_w(0, eng=nc.scalar, halves=2)

            # ---- x resident as one big tile; panel-major (span-major) DMA
            # so qkv(0) span 0 can start before the rest of x lands.
            # Panel 0 is split across the SP and ACT issue queues. ----
            xT = res.tile([128, KT * S], bf16, tag="xT")
            xT3 = xT.rearrange("p (kt s) -> p kt s", kt=KT)

            def xs(kt):
                return xT3[:, kt, :]

            sub0 = span >= 256   # head-0 span-0 split into two column halves

            def xdma(sp, off, ln, g, gi, eng):
                src = x_d[gi * 128:(gi + g) * 128,
                          sp * span + off:sp * span + off + ln]
                eng.dma_start(
                    xT3[:, gi:gi + g, sp * span + off:sp * span + off + ln],
                    src.rearrange("(kt p) s -> p kt s", p=128))

            for sp in range(NS):
                if sp == 0 and sub0:
                    # two column chunks, kt-groups alternating issue queues
                    hs = span // 2
                    g = min(4, KT)
                    for ci in range(2):
                        for i, k0 in enumerate(range(0, KT, g)):
                            eng = nc.scalar if i % 2 == 1 else nc.sync
                            xdma(sp, ci * hs, hs, g, k0, eng)
                else:
                    g = min(4, KT)
                    for k0 in range(0, KT, g):
                        xdma(sp, 0, span, g, k0, nc.sync)

            # ---- persistent per-head outputs ----
            OT = [res.tile([128, S], bf16, tag=f"ot{h}", name=f"ot{h}")
                  for h in range(nh)]
            dsts = {}
            vblk = {}
            wpt = []

            def qkv_units(h):
                """Units of one span x one projection (KT matmuls + bias)."""
                if h + 1 < nh:
                    issue_w(h + 1)
                if h == nh - 1:
                    for g in range(nh):
                        t = wp_p.tile([128, D], bf16, tag="wpt",
                                      name=f"wp{g}")
                        nc.sync.dma_start(t, wp_d[g * 128:(g + 1) * 128, :])
                        wpt.append(t)
                QT = qk_p.tile([128, S], bf16, tag="qt", name=f"qt{h}")
                KTt = qk_p.tile([128, S], bf16, tag="kt_", name=f"ktt{h}")
                VT = qk_p.tile([128, S], bf16, tag="vt", name=f"vt{h}")
                vh = v_p.tile([128, S], bf16, tag="v", name=f"vh{h}")
                dsts[h] = (QT, KTt)
                vblk[h] = vh
                dst3 = (QT, KTt, VT)
                for sp in range(NS):
                    if h == 0 and sp == 0 and sub0:
                        subs = [(0, span // 2), (span // 2, span - span // 2)]
                    else:
                        subs = [(0, span)]
                    for off, ln in subs:
                        for p in range(3):
                            ps = ps_mm.tile([128, span], f32, tag="mm")
                            w = wt[(h, p)]
                            lo = sp * span + off
                            for kt in range(KT):
                                nc.tensor.matmul(
                                    ps[:, :ln], w[:, kt * 128:(kt + 1) * 128],
                                    xs(kt)[:, lo:lo + ln],
                                    start=(kt == 0), stop=(kt == KT - 1))
                            hp = p * nh + h
                            nc.scalar.activation(
                                dst3[p][:, lo:lo + ln], ps[:, :ln],
                                Act.Identity, bias=bq[:, hp:hp + 1], scale=1.0)
                            if p == 2:
                                # V natural layout via PE transpose
                                pst = ps_st.tile([128, span], bf16, tag="st",
                                                 name=f"pst{h}_{sp}_{off}")
                                for j in range(ln // 128):
                                    tb = (lo + j * 128) // 128
                                    nc.tensor.transpose(
                                        pst[:, j * 128:(j + 1) * 128],
                                        VT[:, tb * 128:(tb + 1) * 128],
                                        identb)
                                nc.vector.tensor_copy(
                                    out=vh[:, lo:lo + ln],
                                    in_=pst[:, :ln])
                            yield ("unit", 16 * ln // span)
                    yield ("done", ("qkv", h, sp))

            def attn_units(h):
                """Units of one k-block (score mm + exp + acc; lagged AV mm).
                Span sp is gated on qkv(h) having emitted span sp."""
                yield ("req", ("qkv", h, 0))
                QT, KTt = dsts[h]
                vh = vblk[h]
                for sp in range(NS):
                    if sp > 0:
                        yield ("req", ("qkv", h, sp))
                    nkj = KPS * (sp + 1)
                    po = ps_o.tile([128, span], f32, tag="o")
                    acc = sm_p.tile([128, span], f32, tag="acc")
                    pend = []

                    def av(it):
                        kj, pt, qoff = it
                        nc.tensor.matmul(
                            po[:, qoff:], vh[:, kj * 128:(kj + 1) * 128],
                            pt[:, qoff:],
                            start=(kj == 0), stop=(kj == nkj - 1))

                    for kj in range(nkj):
                        while len(pend) > LAG:
                            av(pend.pop(0))
                        qoff = max(0, kj - KPS * sp) * 128
                        ps = ps_st.tile([128, span], f32, tag="st")
                        nc.tensor.matmul(
                            ps[:, qoff:], KTt[:, kj * 128:(kj + 1) * 128],
                            QT[:, sp * span + qoff:(sp + 1) * span],
                            start=True, stop=True)
                        if kj >= KPS * sp:  # diagonal block: causal mask
                            nc.vector.tensor_tensor(
                                out=ps[:, qoff:qoff + 128],
                                in0=ps[:, qoff:qoff + 128],
                                in1=trimaskT, op=Alu.add)
                        pt = pt_p.tile([128, span], bf16, tag="pt")
                        nc.scalar.activation(
                            pt[:, qoff:], ps[:, qoff:], Act.Exp, scale=scale)
                        if kj == 0:
                            nc.vector.tensor_copy(out=acc, in_=pt)
                        else:
                            nc.vector.tensor_tensor(
                                out=acc[:, qoff:], in0=acc[:, qoff:],
                                in1=pt[:, qoff:], op=Alu.add)
                        pend.append((kj, pt, qoff))
                        yield ("unit", 2)
                    while pend:
                        av(pend.pop(0))
                    # softmax denominator entirely off the PE
                    sumb = sm_p.tile([128, span], f32, tag="sum")
                    nc.gpsimd.partition_all_reduce(
                        sumb, acc, channels=128,
                        reduce_op=bass_isa.ReduceOp.add)
                    recipb = sm_p.tile([128, span], f32, tag="recipb")
                    nc.vector.reciprocal_approx_fast(out=recipb, in_=sumb)
                    nc.vector.tensor_tensor(
                        out=OT[h][:, sp * span:(sp + 1) * span],
                        in0=po, in1=recipb, op=Alu.mult)
                    yield ("unit", 1)
                    yield ("done", ("attn", h, sp))

            def proj_units():
                """Output projection; span sp gated on attn(nh-1) span sp."""
                for sp in range(NS):
                    yield ("req", ("attn", nh - 1, sp))
                    for dcp in range(DC // 2):
                        yc = yc_p.tile([128, 2 * span], f32, tag="yc",
                                       name=f"yc{sp}_{dcp}")
                        last = (sp == NS - 1 and dcp == DC // 2 - 1)
                        for half in range(2):
                            dc = dcp * 2 + half
                            ps = ps_mm.tile([128, span], f32, tag="mm")
                            for g in range(nh):
                                nc.tensor.matmul(
                                    ps, wpt[g][:, dc * 128:(dc + 1) * 128],
                                    OT[g][:, sp * span:(sp + 1) * span],
                                    start=(g == 0), stop=(g == nh - 1))
                            hs = slice(half * span, (half + 1) * span)
                            if half == 0:
                                nc.scalar.copy(yc[:, hs], ps)
                            else:
                                nc.vector.tensor_copy(out=yc[:, hs], in_=ps)
                            if last:
                                # per-half DMA to shrink the kernel tail
                                nc.sync.dma_start(
                                    yt_d[dc * 128:(dc + 1) * 128,
                                         sp * span:(sp + 1) * span],
                                    yc[:, hs])
                            elif half == 1:
                                dst = yt_d[dcp * 256:(dcp + 1) * 256,
                                           sp * span:(sp + 1) * span]
                                nc.sync.dma_start(
                                    dst.rearrange("(two p) s -> p two s",
                                                  p=128),
                                    yc.rearrange("p (two s) -> p two s",
                                                 two=2))
                            yield ("unit", 4)

            # ---- unified dependency-gated two-stream scheduler ----
            def chain(gens):
                for g in gens:
                    yield from g

            def drive(dense, dtot, attn, atot, lead=1.6):
                streams = [
                    {"g": dense, "tot": float(dtot), "w": 0.0,
                     "req": None, "done": False},
                    {"g": attn, "tot": float(atot) * lead, "w": 0.0,
                     "req": None, "done": False},
                ]
                state = set()
                while True:
                    cands = [s for s in streams if not s["done"] and
                             (s["req"] is None or s["req"] in state)]
                    if not cands:
                        if all(s["done"] for s in streams):
                            return
                        raise AssertionError("scheduler deadlock")
                    s = min(cands, key=lambda s: s["w"] / s["tot"])
                    s["req"] = None
                    while True:
                        try:
                            item = next(s["g"])
                        except StopIteration:
                            s["done"] = True
                            break
                        kind = item[0]
                        if kind == "unit":
                            s["w"] += item[1]
                            break
                        elif kind == "done":
                            state.add(item[1])
                        elif kind == "req":
                            if item[1] not in state:
                                s["req"] = item[1]
                                break

            dense_tot = 16 * 3 * NS * nh + 4 * DC * NS
            attn_tot = (2 * (KPS * NS * (NS + 1) // 2) + NS) * nh
            drive(chain([qkv_units(h) for h in range(nh)] + [proj_units()]),
                  dense_tot,
                  chain([attn_units(h) for h in range(nh)]),
                  attn_tot)

    nc.finalize()
    return nc


def _prep_core_inputs(x, W_qkv, b_qkv, W_proj, core, S=S, D=D, nh=HPC):
    import ml_dtypes
    bf16 = ml_dtypes.bfloat16
    ngr = NCORES // B
    b, hg = core // ngr, core % ngr
    KT = D // 128
    Dfull = W_qkv.shape[0]

    wq = np.empty((3 * nh * 128, D), dtype=bf16)
    bq = np.zeros((128, 3 * nh), dtype=np.float32)
    for p in range(3):
        for h in range(nh):
            g = hg * nh + h
            col = p * Dfull + g * 128
            blk = W_qkv[:, col:col + 128]            # [D, 128]
            hp = p * nh + h
            wq[hp * 128:(hp + 1) * 128] = (
                blk.reshape(KT, 128, 128).transpose(1, 0, 2).reshape(128, D)
                .astype(bf16))
            bq[:, hp] = b_qkv[col:col + 128]
    wp = W_proj[hg * nh * 128:(hg + 1) * nh * 128, :].astype(bf16)

    r = np.arange(128)
    trimaskT = np.where(r[:, None] <= r[None, :], 0.0, NEG).astype(np.float32)
    return {
        "xt": np.ascontiguousarray(x[b].T).astype(bf16),
        "wqkv": wq,
        "bqkv": bq,
        "wproj": wp,
        "trimaskT": trimaskT,
        "identb": np.eye(128, dtype=bf16),
    }


_CACHE = {}


def kernel(x, W_qkv, b_qkv, W_proj, b_proj, mask):
    from concourse.bass_utils import run_bass_kernel_spmd

    x = np.asarray(x)
    W_qkv = np.asarray(W_qkv)
    b_qkv = np.asarray(b_qkv)
    W_proj = np.asarray(W_proj)
    b_proj = np.asarray(b_proj)

    if "nc" not in _CACHE:
        _CACHE["nc"] = build_nc()
    nc = _CACHE["nc"]

    in_maps = [_prep_core_inputs(x, W_qkv, b_qkv, W_proj, c)
               for c in range(NCORES)]
    res = run_bass_kernel_spmd(nc, in_maps, core_ids=list(range(NCORES)))

    ngr = NCORES // B
    out = np.empty((B, S, D), dtype=np.float32)
    for b in range(B):
        acc = res.results[b * ngr]["yt"].astype(np.float32)
        for g in range(1, ngr):
            acc = acc + res.results[b * ngr + g]["yt"]
        out[b] = acc.T + b_proj[None, :]
    return out


# revision 25
# speedup vs baseline: 1.0058x; 1.0058x over previous
"""Causal self-attention Trainium2 kernel (8 NeuronCores, bf16 compute).

Sharding: core c -> batch b = c//4, head group hg = c%4 (4 heads each).
Each core computes its heads' QKV projections, causal attention, and a
partial output projection yt[d, t] (transposed). Host sums the 4 partials
per batch, transposes, and adds b_proj.

Device dataflow per core (software-pipelined across heads):
  qkv(h) : per span/proj: PSUM = W.T @ xT chunks -> ACT bias -> QT/KT/VT
           VT 128-blocks transposed to natural V via DMA XBAR transpose
  attn(h): per q-span (512): for each k-block kj:
             ST[k,q] = KT_blk.T @ QT_span   (PE, scores transposed)
             += causal mask on diagonal blocks (DVE)
             PT = exp(scale*ST)             (ACT, bf16, unnormalized)
             acc[128,q] += PT               (DVE, f32)
             OT[hd,q] += V_blk.T @ PT       (PE, lagged)
           sum = partition_all_reduce(acc)  (GPSIMD)
           recip (DVE), OT_sbuf = OT * recip (DVE, bf16)
  proj   : yt[dc,t] = sum_h Wp_h.T @ OT_h -> chunked DMA out
  Interleave: attn(h-1) units are woven between qkv(h) units, and
  attn(3) between proj units, so the ACT-bound exp chain never stalls
  the PE.
"""
import numpy as np

B, S, D, H = 2, 2048, 2048, 16
HD = 128
NCORES = 8
HPC = H // (NCORES // B)     # heads per core = 4
NEG = -1e9


def build_nc(S=S, D=D, nh=HPC, span=512):
    import concourse.mybir as mybir
    from concourse import bacc
    from concourse import bass_isa
    from concourse.tile import TileContext

    f32 = mybir.dt.float32
    bf16 = mybir.dt.bfloat16
    KT = D // 128          # contraction tiles for qkv
    TT = S // 128          # token tiles
    NS = S // span         # q spans
    KPS = span // 128      # k-blocks per span
    DC = D // 128
    scale = float(HD) ** -0.5
    LAG = 2

    nc = bacc.Bacc("TRN2", target_bir_lowering=False, debug=False)
    x_d = nc.dram_tensor("xt", [D, S], bf16, kind="ExternalInput").ap()
    wq_d = nc.dram_tensor("wqkv", [3 * nh * 128, D], bf16,
                          kind="ExternalInput").ap()
    bq_d = nc.dram_tensor("bqkv", [128, 3 * nh], f32, kind="ExternalInput").ap()
    wp_d = nc.dram_tensor("wproj", [nh * 128, D], bf16,
                          kind="ExternalInput").ap()
    tm_d = nc.dram_tensor("trimaskT", [128, 128], f32,
                          kind="ExternalInput").ap()
    id_d = nc.dram_tensor("identb", [128, 128], bf16, kind="ExternalInput").ap()
    yt_d = nc.dram_tensor("yt", [D, S], f32, kind="ExternalOutput").ap()

    Act = mybir.ActivationFunctionType
    Alu = mybir.AluOpType

    with TileContext(nc) as tc:
        from contextlib import ExitStack
        with ExitStack() as ctx:
            res = ctx.enter_context(tc.tile_pool(name="res", bufs=1))
            w_p = ctx.enter_context(tc.tile_pool(name="w", bufs=6))
            wp_p = ctx.enter_context(tc.tile_pool(name="wp", bufs=nh))
            qk_p = ctx.enter_context(tc.tile_pool(name="qk", bufs=2))
            v_p = ctx.enter_context(tc.tile_pool(name="v", bufs=2))
            pt_p = ctx.enter_context(tc.tile_pool(name="pt", bufs=4))
            sm_p = ctx.enter_context(tc.tile_pool(name="sm", bufs=2))
            yc_p = ctx.enter_context(tc.tile_pool(name="yc", bufs=4))
            ps_mm = ctx.enter_context(
                tc.tile_pool(name="ps_mm", bufs=2, space="PSUM"))
            ps_o = ctx.enter_context(
                tc.tile_pool(name="ps_o", bufs=2, space="PSUM"))
            ps_st = ctx.enter_context(
                tc.tile_pool(name="ps_st", bufs=4, space="PSUM"))

            # constants (issued on the ACT queue; SP queue is for x panels)
            trimaskT = res.tile([128, 128], f32, tag="trimaskT")
            identb = res.tile([128, 128], bf16, tag="identb")
            bq = res.tile([128, 3 * nh], f32, tag="bq")
            nc.scalar.dma_start(trimaskT, tm_d)
            nc.scalar.dma_start(identb, id_d)
            nc.scalar.dma_start(bq, bq_d)

            wt = {}

            def issue_w(h, eng=None, halves=1):
                for p in range(3):
                    t = w_p.tile([128, D], bf16, tag="w", name=f"w{h}_{p}")
                    r0 = (p * nh + h) * 128
                    hD = D // halves
                    for q in range(halves):
                        (eng or nc.sync).dma_start(
                            t[:, q * hD:(q + 1) * hD],
                            wq_d[r0:r0 + 128, q * hD:(q + 1) * hD])
                    wt[(h, p)] = t

            issue_w(0, eng=nc.scalar, halves=2)

            # ---- x resident as one big tile; panel-major (span-major) DMA
            # so qkv(0) span 0 can start before the rest of x lands.
            # Panel 0 is split across the SP and ACT issue queues. ----
            xT = res.tile([128, KT * S], bf16, tag="xT")
            xT3 = xT.rearrange("p (kt s) -> p kt s", kt=KT)

            def xs(kt):
                return xT3[:, kt, :]

            sub0 = False   # head-0 span-0 column-split (not profitable)

            def xdma(sp, off, ln, g, gi, eng):
                src = x_d[gi * 128:(gi + g) * 128,
                          sp * span + off:sp * span + off + ln]
                eng.dma_start(
                    xT3[:, gi:gi + g, sp * span + off:sp * span + off + ln],
                    src.rearrange("(kt p) s -> p kt s", p=128))

            for sp in range(NS):
                g = min(2 if sp == 0 else 4, KT)
                for i, k0 in enumerate(range(0, KT, g)):
                    eng = nc.scalar if (sp == 0 and i % 2 == 1) else nc.sync
                    xdma(sp, 0, span, g, k0, eng)

            # ---- persistent per-head outputs ----
            OT = [res.tile([128, S], bf16, tag=f"ot{h}", name=f"ot{h}")
                  for h in range(nh)]
            dsts = {}
            vblk = {}
            wpt = []

            def qkv_units(h):
                """Units of one span x one projection (KT matmuls + bias)."""
                if h == nh - 1:
                    for g in range(nh):
                        t = wp_p.tile([128, D], bf16, tag="wpt",
                                      name=f"wp{g}")
                        nc.sync.dma_start(t, wp_d[g * 128:(g + 1) * 128, :])
                        wpt.append(t)
                QT = qk_p.tile([128, S], bf16, tag="qt", name=f"qt{h}")
                KTt = qk_p.tile([128, S], bf16, tag="kt_", name=f"ktt{h}")
                VT = qk_p.tile([128, S], bf16, tag="vt", name=f"vt{h}")
                vh = v_p.tile([128, S], bf16, tag="v", name=f"vh{h}")
                dsts[h] = (QT, KTt)
                vblk[h] = vh
                dst3 = (QT, KTt, VT)
                for sp in range(NS):
                    if h == 0 and sp == 0 and sub0:
                        subs = [(0, span // 2), (span // 2, span - span // 2)]
                    else:
                        subs = [(0, span)]
                    for off, ln in subs:
                        for p in range(3):
                            ps = ps_mm.tile([128, span], f32, tag="mm")
                            w = wt[(h, p)]
                            lo = sp * span + off
                            for kt in range(KT):
                                nc.tensor.matmul(
                                    ps[:, :ln], w[:, kt * 128:(kt + 1) * 128],
                                    xs(kt)[:, lo:lo + ln],
                                    start=(kt == 0), stop=(kt == KT - 1))
                            hp = p * nh + h
                            nc.scalar.activation(
                                dst3[p][:, lo:lo + ln], ps[:, :ln],
                                Act.Identity, bias=bq[:, hp:hp + 1], scale=1.0)
                            if p == 2:
                                # V natural layout via PE transpose
                                pst = ps_st.tile([128, span], bf16, tag="st",
                                                 name=f"pst{h}_{sp}_{off}")
                                for j in range(ln // 128):
                                    tb = (lo + j * 128) // 128
                                    nc.tensor.transpose(
                                        pst[:, j * 128:(j + 1) * 128],
                                        VT[:, tb * 128:(tb + 1) * 128],
                                        identb)
                                nc.vector.tensor_copy(
                                    out=vh[:, lo:lo + ln],
                                    in_=pst[:, :ln])
                            yield ("unit", 16 * ln // span)
                    yield ("done", ("qkv", h, sp))
                    if sp == min(1, NS - 1) and h + 1 < nh:
                        # prefetch next head's weights after the early x
                        # panels have had the DMA bandwidth to themselves
                        issue_w(h + 1)

            def attn_units(h):
                """Units of one k-block (score mm + exp + acc; lagged AV mm).
                Span sp is gated on qkv(h) having emitted span sp."""
                yield ("req", ("qkv", h, 0))
                QT, KTt = dsts[h]
                vh = vblk[h]
                for sp in range(NS):
                    if sp > 0:
                        yield ("req", ("qkv", h, sp))
                    nkj = KPS * (sp + 1)
                    po = ps_o.tile([128, span], f32, tag="o")
                    acc = sm_p.tile([128, span], f32, tag="acc")
                    pend = []

                    def av(it):
                        kj, pt, qoff = it
                        nc.tensor.matmul(
                            po[:, qoff:], vh[:, kj * 128:(kj + 1) * 128],
                            pt[:, qoff:],
                            start=(kj == 0), stop=(kj == nkj - 1))

                    for kj in range(nkj):
                        while len(pend) > LAG:
                            av(pend.pop(0))
                        qoff = max(0, kj - KPS * sp) * 128
                        ps = ps_st.tile([128, span], f32, tag="st")
                        nc.tensor.matmul(
                            ps[:, qoff:], KTt[:, kj * 128:(kj + 1) * 128],
                            QT[:, sp * span + qoff:(sp + 1) * span],
                            start=True, stop=True)
                        if kj >= KPS * sp:  # diagonal block: causal mask
                            nc.vector.tensor_tensor(
                                out=ps[:, qoff:qoff + 128],
                                in0=ps[:, qoff:qoff + 128],
                                in1=trimaskT, op=Alu.add)
                        pt = pt_p.tile([128, span], bf16, tag="pt")
                        nc.scalar.activation(
                            pt[:, qoff:], ps[:, qoff:], Act.Exp, scale=scale)
                        if kj == 0:
                            nc.vector.tensor_copy(out=acc, in_=pt)
                        else:
                            nc.vector.tensor_tensor(
                                out=acc[:, qoff:], in0=acc[:, qoff:],
                                in1=pt[:, qoff:], op=Alu.add)
                        pend.append((kj, pt, qoff))
                        yield ("unit", 2)
                    while pend:
                        av(pend.pop(0))
                    # softmax denominator entirely off the PE. bf16 input
                    # halves the GPSIMD read; quantization of the partial
                    # sums averages out ~sqrt(128) in the reduction.
                    accb = sm_p.tile([128, span], bf16, tag="accb")
                    nc.vector.tensor_copy(out=accb, in_=acc)
                    sumb = sm_p.tile([128, span], f32, tag="sum")
                    nc.gpsimd.partition_all_reduce(
                        sumb, accb, channels=128,
                        reduce_op=bass_isa.ReduceOp.add)
                    recipb = sm_p.tile([128, span], f32, tag="recipb")
                    nc.vector.reciprocal_approx_fast(out=recipb, in_=sumb)
                    nc.vector.tensor_tensor(
                        out=OT[h][:, sp * span:(sp + 1) * span],
                        in0=po, in1=recipb, op=Alu.mult)
                    yield ("unit", 1)
                    yield ("done", ("attn", h, sp))

            def proj_units():
                """Output projection; span sp gated on attn(nh-1) span sp."""
                for sp in range(NS):
                    yield ("req", ("attn", nh - 1, sp))
                    for dcp in range(DC // 2):
                        yc = yc_p.tile([128, 2 * span], f32, tag="yc",
                                       name=f"yc{sp}_{dcp}")
                        last = (sp == NS - 1 and dcp == DC // 2 - 1)
                        for half in range(2):
                            dc = dcp * 2 + half
                            ps = ps_mm.tile([128, span], f32, tag="mm")
                            for g in range(nh):
                                nc.tensor.matmul(
                                    ps, wpt[g][:, dc * 128:(dc + 1) * 128],
                                    OT[g][:, sp * span:(sp + 1) * span],
                                    start=(g == 0), stop=(g == nh - 1))
                            hs = slice(half * span, (half + 1) * span)
                            if half == 0:
                                nc.scalar.copy(yc[:, hs], ps)
                            else:
                                nc.vector.tensor_copy(out=yc[:, hs], in_=ps)
                            if last:
                                # per-half DMA to shrink the kernel tail
                                nc.sync.dma_start(
                                    yt_d[dc * 128:(dc + 1) * 128,
                                         sp * span:(sp + 1) * span],
                                    yc[:, hs])
                            elif half == 1:
                                dst = yt_d[dcp * 256:(dcp + 1) * 256,
                                           sp * span:(sp + 1) * span]
                                nc.sync.dma_start(
                                    dst.rearrange("(two p) s -> p two s",
                                                  p=128),
                                    yc.rearrange("p (two s) -> p two s",
                                                 two=2))
                            yield ("unit", 4)

            # ---- unified dependency-gated two-stream scheduler ----
            def chain(gens):
                for g in gens:
                    yield from g

            def drive(dense, dtot, attn, atot, lead=1.6):
                streams = [
                    {"g": dense, "tot": float(dtot), "w": 0.0,
                     "req": None, "done": False},
                    {"g": attn, "tot": float(atot) * lead, "w": 0.0,
                     "req": None, "done": False},
                ]
                state = set()
                while True:
                    cands = [s for s in streams if not s["done"] and
                             (s["req"] is None or s["req"] in state)]
                    if not cands:
                        if all(s["done"] for s in streams):
                            return
                        raise AssertionError("scheduler deadlock")
                    s = min(cands, key=lambda s: s["w"] / s["tot"])
                    s["req"] = None
                    while True:
                        try:
                            item = next(s["g"])
                        except StopIteration:
                            s["done"] = True
                            break
                        kind = item[0]
                        if kind == "unit":
                            s["w"] += item[1]
                            break
                        elif kind == "done":
                            state.add(item[1])
                        elif kind == "req":
                            if item[1] not in state:
                                s["req"] = item[1]
                                break

            dense_tot = 16 * 3 * NS * nh + 4 * DC * NS
            attn_tot = (2 * (KPS * NS * (NS + 1) // 2) + NS) * nh
            drive(chain([qkv_units(h) for h in range(nh)] + [proj_units()]),
                  dense_tot,
                  chain([attn_units(h) for h in range(nh)]),
                  attn_tot)

    nc.finalize()
    return nc


def _prep_core_inputs(x, W_qkv, b_qkv, W_proj, core, S=S, D=D, nh=HPC):
    import ml_dtypes
    bf16 = ml_dtypes.bfloat16
    ngr = NCORES // B
    b, hg = core // ngr, core % ngr
    KT = D // 128
    Dfull = W_qkv.shape[0]

    wq = np.empty((3 * nh * 128, D), dtype=bf16)
    bq = np.zeros((128, 3 * nh), dtype=np.float32)
    for p in range(3):
        for h in range(nh):
            g = hg * nh + h
            col = p * Dfull + g * 128
            blk = W_qkv[:, col:col + 128]            # [D, 128]
            hp = p * nh + h
            wq[hp * 128:(hp + 1) * 128] = (
                blk.reshape(KT, 128, 128).transpose(1, 0, 2).reshape(128, D)
                .astype(bf16))
            bq[:, hp] = b_qkv[col:col + 128]
    wp = W_proj[hg * nh * 128:(hg + 1) * nh * 128, :].astype(bf16)

    r = np.arange(128)
    trimaskT = np.where(r[:, None] <= r[None, :], 0.0, NEG).astype(np.float32)
    return {
        "xt": np.ascontiguousarray(x[b].T).astype(bf16),
        "wqkv": wq,
        "bqkv": bq,
        "wproj": wp,
        "trimaskT": trimaskT,
        "identb": np.eye(128, dtype=bf16),
    }


_CACHE = {}


def kernel(x, W_qkv, b_qkv, W_proj, b_proj, mask):
    from concourse.bass_utils import run_bass_kernel_spmd

    x = np.asarray(x)
    W_qkv = np.asarray(W_qkv)
    b_qkv = np.asarray(b_qkv)
    W_proj = np.asarray(W_proj)
    b_proj = np.asarray(b_proj)

    if "nc" not in _CACHE:
        _CACHE["nc"] = build_nc()
    nc = _CACHE["nc"]

    in_maps = [_prep_core_inputs(x, W_qkv, b_qkv, W_proj, c)
               for c in range(NCORES)]
    res = run_bass_kernel_spmd(nc, in_maps, core_ids=list(range(NCORES)))

    ngr = NCORES // B
    out = np.empty((B, S, D), dtype=np.float32)
    for b in range(B):
        acc = res.results[b * ngr]["yt"].astype(np.float32)
        for g in range(1, ngr):
            acc = acc + res.results[b * ngr + g]["yt"]
        out[b] = acc.T + b_proj[None, :]
    return out
